# revision 1
# baseline (speedup 1.0000x reference)
"""2-layer GAT (GATConv x2 + LayerNorm + ReLU) on Trainium2, 8-core SPMD.

Strategy (graph/data parallel, per the sharding hint):
  - Nodes are permuted (degree-sorted, dealt round-robin) and sharded across
    8 cores by destination node; each core owns TILES*128 dst slots.
  - Per layer a packed bf16 node table is built on device:
      layer 1: [h~1(128) | a_s1(4) | a_d1(4) | pad]  (256 cols, 512B rows)
      layer 2: [h~2(64)  | a_s2(1) | a_d2(1) | pad]  (128 cols, 256B rows)
    Layer-1 tables are computed replicated (full x @ W1 on every core);
    layer-2 tables are computed per-shard and exchanged with AllGather.
  - Per dst tile [128 dst x K slots] neighbor rows are fetched with
    dma_gather (int16 indices; two half-tables to fit the int16 range; two
    extra "self" slots carry a_d[dst]).  Padded slots point at rows whose
    a_s is -1e9 so they vanish in the softmax.
  - Segment softmax over the padded K dimension and the weighted message
    aggregation run as strided DVE ops (mult + strided reduction).
"""

import os
import types
from contextlib import ExitStack

import numpy as np

import concourse.bass as bass
import concourse.mybir as mybir
import concourse.tile as tile
from concourse import bacc
from concourse.bass import AP
from concourse.masks import make_identity

F32 = mybir.dt.float32
BF16 = mybir.dt.bfloat16
I16 = mybir.dt.int16
AX = mybir.AxisListType
OP = mybir.AluOpType
ACT = mybir.ActivationFunctionType

# ---------------------------------------------------------------- problem cfg
N = 50000
E = 800000
IN_DIM = 128
HID = 32
HEADS = 4
EMB = 64
NEG = 0.2
EPS = 1e-5
NCORE = 8
TB1 = 256        # packed table-1 row (bf16 cols): h(128) as(4) ad(4) pad
TB2 = 128        # packed table-2 row: h(64) as(1) ad(1) pad
NEGBIG = -1e9


def make_cfg(n_nodes, tiles_per_core):
    c = types.SimpleNamespace()
    c.N = n_nodes
    c.TILES = tiles_per_core
    c.NPC = tiles_per_core * 128
    c.NPAD = NCORE * c.NPC
    c.SPLIT = c.NPAD // 2
    c.REAL_PC = n_nodes // NCORE
    c.PADA = c.REAL_PC                      # -1e9 row in half A (core 0 pad)
    c.PADB = c.REAL_PC                      # -1e9 row in half B (core 4 pad,
    #                                         value relative to SPLIT)
    assert c.REAL_PC < c.NPC and c.SPLIT <= 32768 and n_nodes % NCORE == 0
    return c


CFG = make_cfg(N, 49)


# ------------------------------------------------------------------ host prep
def host_prep(cfg, edge_index):
    """Permutation, per-tile K sizes and the int16 gather index stream.

    Per tile the index stream holds [KA edge cols | 1 self col] for half A
    then [KB edge cols | 1 self col] for half B (each column = 128 slots).
    """
    n, npc, tiles, npad, split = cfg.N, cfg.NPC, cfg.TILES, cfg.NPAD, cfg.SPLIT
    src = np.concatenate([edge_index[0], np.arange(n, dtype=np.int64)])
    dst = np.concatenate([edge_index[1], np.arange(n, dtype=np.int64)])

    deg = np.bincount(dst, minlength=n)
    order = np.argsort(-deg, kind="stable")
    newid = np.empty(n, np.int64)
    r = np.arange(n)
    newid[order] = (r % NCORE) * npc + (r // NCORE)
    new2old = np.full(npad, -1, np.int64)
    new2old[newid] = np.arange(n)

    sn = newid[src]
    dn = newid[dst]
    lo = sn < split

    cnt_lo = np.bincount(dn[lo], minlength=npad).reshape(NCORE, tiles, 128)
    cnt_hi = np.bincount(dn[~lo], minlength=npad).reshape(NCORE, tiles, 128)
    KA = cnt_lo.max(axis=(0, 2)).astype(np.int64)
    KB = cnt_hi.max(axis=(0, 2)).astype(np.int64)

    key = dn * 2 + (~lo)
    eo = np.argsort(key, kind="stable")
    ks = key[eo]
    starts = np.r_[0, np.flatnonzero(np.diff(ks)) + 1]
    runlen = np.diff(np.r_[starts, len(ks)])
    runpos = np.arange(len(ks)) - np.repeat(starts, runlen)
    sne, dne, loe = sn[eo], dn[eo], lo[eo]
    ce = dne // npc
    je = dne % npc
    te = je // 128
    pe = je % 128

    blk = (KA + KB + 2) * 128
    off = np.r_[0, np.cumsum(blk)]
    total16 = int(off[-1])
    offA = off[:-1]                         # A edges start
    selfA = offA + KA * 128                 # A self column
    offB = selfA + 128                      # B edges start
    selfB = offB + KB * 128                 # B self column
    idx16 = np.full((NCORE, total16), cfg.PADA, np.int16)   # pads -> -1e9 row
    pos = np.where(loe, offA[te] + runpos * 128 + pe,
                   offB[te] + runpos * 128 + pe)
    val = np.where(loe, sne, sne - split).astype(np.int16)
    idx16[ce, pos] = val
    # self columns: own row in the half that contains this core's shard;
    # the other half's self column points at a zero-a_d row (the pad row).
    pp = np.arange(128)[None, :]
    for c in range(NCORE):
        for t in range(tiles):
            own = c * npc + t * 128 + np.arange(128)
            vA = own if c < NCORE // 2 else np.full(128, cfg.PADA)
            vB = (own - split) if c >= NCORE // 2 else np.full(128, cfg.PADB)
            idx16[c, selfA[t] + np.arange(128)] = vA.astype(np.int16)
            idx16[c, selfB[t] + np.arange(128)] = vB.astype(np.int16)

    w = idx16.reshape(NCORE, total16 // 16, 16).transpose(0, 2, 1)
    idx16_w = np.tile(w, (1, 8, 1))
    return types.SimpleNamespace(
        new2old=new2old, newid=newid,
        KA=KA.astype(int), KB=KB.astype(int), K2=(KA + KB + 2).astype(int),
        colA=(offA // 16).astype(int), colB=(offB // 16).astype(int),
        c16=total16 // 16, idx16=idx16_w,
    )


def host_weights(cfg, inputs):
    W1 = np.asarray(inputs["W1"], np.float32)
    W2 = np.asarray(inputs["W2"], np.float32)
    as1 = np.asarray(inputs["att_src1"], np.float32)
    ad1 = np.asarray(inputs["att_dst1"], np.float32)
    as2 = np.asarray(inputs["att_src2"], np.float32)
    ad2 = np.asarray(inputs["att_dst2"], np.float32)
    W1r = W1.reshape(IN_DIM, HEADS, HID)
    w_as1 = np.einsum("fhc,hc->fh", W1r, as1)
    w_ad1 = np.einsum("fhc,hc->fh", W1r, ad1)
    W1ext = np.concatenate([W1, w_as1, w_ad1], axis=1)            # [128,136]
    W2ext = np.concatenate([W2, W2 @ as2[0][:, None], W2 @ ad2[0][:, None]],
                           axis=1)                                # [128,66]
    return {
        "W1ext": np.ascontiguousarray(W1ext),
        "W2ext": np.ascontiguousarray(W2ext),
        "B1": np.tile(np.asarray(inputs["b1"], np.float32), (128, 1)),
        "G1": np.tile(np.asarray(inputs["gamma1"], np.float32), (128, 1)),
        "Be1": np.tile(np.asarray(inputs["beta1"], np.float32), (128, 1)),
        "B2": np.tile(np.asarray(inputs["b2"], np.float32), (128, 1)),
        "G2": np.tile(np.asarray(inputs["gamma2"], np.float32), (128, 1)),
        "Be2": np.tile(np.asarray(inputs["beta2"], np.float32), (128, 1)),
    }


def host_xt(cfg, prep, x):
    xt = np.zeros((IN_DIM, cfg.NPAD), np.float32)
    xt[:, prep.newid] = np.asarray(x, np.float32).T
    return xt


# ----------------------------------------------------------------- AP helpers
def apv(ap: AP, dims):
    """Replace the free dims of `ap` with explicit [step, count] pairs."""
    return AP(ap.tensor, ap.offset, [list(ap.ap[0])] + [list(d) for d in dims])


# ------------------------------------------------------------- device program
def build_program(cfg, prep):
    maxphase = int(os.environ.get("GAT_MAXPHASE", "4"))
    nc = bacc.Bacc("TRN2", target_bir_lowering=False, debug=False,
                   num_devices=NCORE)
    tiles, npc, npad, split = cfg.TILES, cfg.NPC, cfg.NPAD, cfg.SPLIT
    KA, KB, K2 = prep.KA, prep.KB, prep.K2
    FB1 = IN_DIM + 2 * HEADS            # 136
    FB2 = EMB + 2                       # 66

    XT = nc.dram_tensor("xt", [IN_DIM, npad], F32, kind="ExternalInput")
    W1e = nc.dram_tensor("w1ext", [IN_DIM, FB1], F32, kind="ExternalInput")
    W2e = nc.dram_tensor("w2ext", [IN_DIM, FB2], F32, kind="ExternalInput")
    IDX16 = nc.dram_tensor("idx16", [128, prep.c16], I16, kind="ExternalInput")
    CB = {}
    for nm, cols in [("B1", IN_DIM), ("G1", IN_DIM), ("Be1", IN_DIM),
                     ("B2", EMB), ("G2", EMB), ("Be2", EMB)]:
        CB[nm] = nc.dram_tensor(nm.lower(), [128, cols], F32,
                                kind="ExternalInput")
    OUT = nc.dram_tensor("out", [npc, EMB], F32, kind="ExternalOutput")

    with tile.TileContext(nc, num_cores=NCORE) as tc, ExitStack() as ctx:
        dram = ctx.enter_context(tc.tile_pool(name="dram", bufs=1,
                                              space="DRAM"))
        t1b = dram.tile([npad, TB1], BF16, name="t1b")
        t2sh = dram.tile([npc, TB2], BF16, name="t2sh")
        t2b = dram.tile([npad, TB2], BF16, name="t2b")

        cpool = ctx.enter_context(tc.tile_pool(name="const", bufs=1))
        w1s = cpool.tile([IN_DIM, FB1], F32, name="w1s")
        w2s = cpool.tile([IN_DIM, FB2], F32, name="w2s")
        nc.sync.dma_start(w1s[:], W1e[:])
        nc.sync.dma_start(w2s[:], W2e[:])
        cb = {}
        for nm in CB:
            cb[nm] = cpool.tile(list(CB[nm].shape), F32, name=f"sb_{nm}")
            nc.sync.dma_start(cb[nm][:], CB[nm][:])
        ident = cpool.tile([128, 128], F32, name="ident")
        make_identity(nc, ident[:])
        negb = cpool.tile([32, 8], BF16, name="negb")
        nc.vector.memset(negb[:], NEGBIG)
        zerb = cpool.tile([32, 8], BF16, name="zerb")
        nc.vector.memset(zerb[:], 0.0)
        epst = cpool.tile([128, 1], F32, name="epst")
        nc.vector.memset(epst[:], EPS)
        i16b = cpool.tile([128, prep.c16], I16, name="i16b")
        nc.sync.dma_start(i16b[:], IDX16[:])
        if maxphase < 4:
            dummy = cpool.tile([128, EMB], F32, name="dummy")
            nc.vector.memset(dummy[:], 0.5)
            nc.sync.dma_start(OUT[0:128, :], dummy[:])

        # ---------------- phase 1: layer-1 packed table, replicated
        GRP = 4
        ngrp = npad // (128 * GRP)
        with tc.tile_pool(name="ph1", bufs=3) as ph1, \
             tc.tile_pool(name="ph1p", bufs=8, space="PSUM") as ph1p:
            for g in range(ngrp):
                xsl = ph1.tile([128, 128 * GRP], F32, tag="xsl")
                nc.sync.dma_start(xsl[:], XT[:, g * 128 * GRP:(g + 1) * 128 * GRP])
                stage = ph1.tile([128, GRP, FB1], BF16, tag="stage")
                for s in range(GRP):
                    ps = ph1p.tile([128, FB1], F32, tag="ps")
                    nc.tensor.matmul(ps[:], lhsT=xsl[:, s * 128:(s + 1) * 128],
                                     rhs=w1s[:], start=True, stop=True)
                    nc.vector.tensor_copy(stage[:, s, :], ps[:])
                rows = slice(g * 128 * GRP, (g + 1) * 128 * GRP)
                nc.sync.dma_start(
                    t1b[rows, 0:FB1].rearrange("(s p) c -> p s c", p=128),
                    stage[:, :, :])
        # pad rows: a_s1 = -1e9 in both halves
        pr = min(32, npc - cfg.REAL_PC)
        nc.sync.dma_start(t1b[cfg.PADA:cfg.PADA + pr, IN_DIM:IN_DIM + HEADS],
                          negb[0:pr, 0:HEADS])
        nc.sync.dma_start(
            t1b[split + cfg.PADB:split + cfg.PADB + pr, IN_DIM:IN_DIM + HEADS],
            negb[0:pr, 0:HEADS])

        # ---------------- phase 2+3: layer-1 per-tile; build layer-2 shard
        with tc.tile_pool(name="gp", bufs=2) as gp, \
             tc.tile_pool(name="sp", bufs=3) as sp, \
             tc.tile_pool(name="pp", bufs=4, space="PSUM") as pp:
            for t in (range(tiles) if maxphase >= 2 else []):
                ka, kb, k2 = KA[t], KB[t], K2[t]
                sA, sB = ka, ka + kb + 1          # self slot positions
                G = gp.tile([128, k2, TB1], BF16, tag="G")
                nc.gpsimd.dma_gather(
                    G[:, 0:ka + 1, :], t1b[0:split, :],
                    i16b[:, prep.colA[t]:prep.colA[t] + (ka + 1) * 8],
                    (ka + 1) * 128, (ka + 1) * 128, TB1, single_packet=False)
                nc.gpsimd.dma_gather(
                    G[:, ka + 1:k2, :], t1b[split:npad, :],
                    i16b[:, prep.colB[t]:prep.colB[t] + (kb + 1) * 8],
                    (kb + 1) * 128, (kb + 1) * 128, TB1, single_packet=False)

                # a_d[dst]: sum of the two self slots (one genuine, the other
                # points at a zero-a_d row)
                ad = sp.tile([128, HEADS], F32, tag="ad")
                nc.vector.tensor_tensor(ad[:], G[:, sA, IN_DIM + HEADS:FB1],
                                        G[:, sB, IN_DIM + HEADS:FB1], OP.add)
                # e = leaky(a_s[src] + a_d[dst]) over all k2 slots
                e = sp.tile([128, k2, HEADS], F32, tag="e")
                e2 = sp.tile([128, k2, HEADS], F32, tag="e2")
                as_v = apv(G[:, :, IN_DIM:IN_DIM + HEADS],
                           [[TB1, k2], [1, HEADS]])
                ad_v = apv(ad[:], [[0, k2], [1, HEADS]])
                nc.vector.tensor_tensor(e[:], as_v, ad_v, OP.add)
                nc.vector.tensor_scalar_mul(e2[:], e[:], NEG)
                nc.vector.tensor_tensor(e[:], e[:], e2[:], OP.max)
                # kill the self slots
                nc.vector.memset(e[:, sA, :], NEGBIG)
                nc.vector.memset(e[:, sB, :], NEGBIG)
                # softmax over k2 per head
                mx = sp.tile([128, HEADS], F32, tag="mx")
                e_hk = apv(e[:], [[1, HEADS], [HEADS, k2]])
                nc.vector.reduce_max(mx[:], e_hk, axis=AX.X)
                mx_b = apv(mx[:], [[0, k2], [1, HEADS]])
                nc.vector.tensor_tensor(e[:], e[:], mx_b, OP.subtract)
                nc.scalar.activation(e[:], e[:], ACT.Exp)
                den = sp.tile([128, HEADS], F32, tag="den")
                nc.vector.reduce_sum(den[:], e_hk, axis=AX.X)
                inv = sp.tile([128, HEADS], F32, tag="inv")
                nc.vector.reciprocal(inv[:], den[:])
                inv_b = apv(inv[:], [[0, k2], [1, HEADS]])
                nc.vector.tensor_tensor(e[:], e[:], inv_b, OP.mult)
                # msg = h * alpha ; aggregate over k2
                gh = apv(G[:], [[TB1, k2], [1, IN_DIM]])
                alpha_b = apv(e[:], [[HEADS, k2], [1, HEADS], [0, HID]])
                nc.vector.tensor_tensor(gh, gh, alpha_b, OP.mult)
                h1 = sp.tile([128, IN_DIM], F32, tag="h1")
                g_fk = apv(G[:], [[1, IN_DIM], [TB1, k2]])
                nc.vector.reduce_sum(h1[:], g_fk, axis=AX.X)

                # + b1, layernorm, relu
                nc.vector.tensor_tensor(h1[:], h1[:], cb["B1"][:], OP.add)
                ms = sp.tile([128, 1], F32, tag="ms")
                nc.vector.reduce_sum(ms[:], h1[:], axis=AX.X)
                mu = sp.tile([128, 1], F32, tag="mu")
                nc.scalar.activation(mu[:], ms[:], ACT.Copy, scale=1.0 / IN_DIM)
                nc.vector.tensor_scalar_sub(h1[:], h1[:], mu[:])
                sq = sp.tile([128, IN_DIM], F32, tag="sq")
                nc.scalar.activation(sq[:], h1[:], ACT.Square)
                var = sp.tile([128, 1], F32, tag="var")
                nc.vector.reduce_sum(var[:], sq[:], axis=AX.X)
                std = sp.tile([128, 1], F32, tag="std")
                nc.scalar.activation(std[:], var[:], ACT.Sqrt, bias=epst[:],
                                     scale=1.0 / IN_DIM)
                rstd = sp.tile([128, 1], F32, tag="rstd")
                nc.vector.reciprocal(rstd[:], std[:])
                nc.vector.tensor_scalar_mul(h1[:], h1[:], rstd[:])
                nc.vector.tensor_tensor(h1[:], h1[:], cb["G1"][:], OP.mult)
                nc.vector.tensor_tensor(h1[:], h1[:], cb["Be1"][:], OP.add)
                nc.vector.tensor_scalar_max(h1[:], h1[:], 0.0)

                # layer-2 packed shard rows: h1 @ W2ext via PE transpose
                pst = pp.tile([128, 128], F32, tag="pst")
                nc.tensor.transpose(pst[:], h1[:], ident[:])
                h1t = sp.tile([128, 128], F32, tag="h1t")
                nc.vector.tensor_copy(h1t[:], pst[:])
                ps2 = pp.tile([128, FB2], F32, tag="ps2")
                nc.tensor.matmul(ps2[:], lhsT=h1t[:], rhs=w2s[:],
                                 start=True, stop=True)
                t2row = sp.tile([128, FB2], BF16, tag="t2row")
                nc.vector.tensor_copy(t2row[:], ps2[:])
                nc.sync.dma_start(t2sh[t * 128:(t + 1) * 128, 0:FB2], t2row[:])

        # pad rows of the shard: a_s2 = -1e9, a_d2 = 0 (pre-collective)
        nc.sync.dma_start(t2sh[cfg.REAL_PC:cfg.REAL_PC + pr, EMB:EMB + 1],
                          negb[0:pr, 0:1])
        nc.sync.dma_start(t2sh[cfg.REAL_PC:cfg.REAL_PC + pr, EMB + 1:EMB + 2],
                          zerb[0:pr, 0:1])
        if maxphase >= 3:
            nc.gpsimd.collective_compute(
                "AllGather", OP.bypass,
                replica_groups=[list(range(NCORE))],
                ins=[t2sh[:].opt()], outs=[t2b[:].opt()])

        # ---------------- phase 4: layer 2
        with tc.tile_pool(name="gp2", bufs=2) as gp2, \
             tc.tile_pool(name="sp2", bufs=3) as sp2:
            for t in (range(tiles) if maxphase >= 4 else []):
                ka, kb, k2 = KA[t], KB[t], K2[t]
                sA, sB = ka, ka + kb + 1
                G = gp2.tile([128, k2, TB2], BF16, tag="G2")
                nc.gpsimd.dma_gather(
                    G[:, 0:ka + 1, :], t2b[0:split, :],
                    i16b[:, prep.colA[t]:prep.colA[t] + (ka + 1) * 8],
                    (ka + 1) * 128, (ka + 1) * 128, TB2, single_packet=False)
                nc.gpsimd.dma_gather(
                    G[:, ka + 1:k2, :], t2b[split:npad, :],
                    i16b[:, prep.colB[t]:prep.colB[t] + (kb + 1) * 8],
                    (kb + 1) * 128, (kb + 1) * 128, TB2, single_packet=False)

                ad = sp2.tile([128, 1], F32, tag="ad2")
                nc.vector.tensor_tensor(ad[:], G[:, sA, EMB + 1:EMB + 2],
                                        G[:, sB, EMB + 1:EMB + 2], OP.add)
                e = sp2.tile([128, k2], F32, tag="e_2")
                e2 = sp2.tile([128, k2], F32, tag="e2_2")
                nc.vector.tensor_scalar_add(e[:], apv(G[:, :, EMB:EMB + 1],
                                                      [[TB2, k2]]), ad[:])
                nc.vector.tensor_scalar_mul(e2[:], e[:], NEG)
                nc.vector.tensor_tensor(e[:], e[:], e2[:], OP.max)
                nc.vector.memset(e[:, sA:sA + 1], NEGBIG)
                nc.vector.memset(e[:, sB:sB + 1], NEGBIG)
                mx = sp2.tile([128, 1], F32, tag="mx2")
                nc.vector.reduce_max(mx[:], e[:], axis=AX.X)
                nc.vector.tensor_scalar_sub(e[:], e[:], mx[:])
                nc.scalar.activation(e[:], e[:], ACT.Exp)
                den = sp2.tile([128, 1], F32, tag="den2")
                nc.vector.reduce_sum(den[:], e[:], axis=AX.X)
                inv = sp2.tile([128, 1], F32, tag="inv2")
                nc.vector.reciprocal(inv[:], den[:])
                nc.vector.tensor_scalar_mul(e[:], e[:], inv[:])
                gh = apv(G[:], [[TB2, k2], [1, EMB]])
                alpha_b = apv(e[:], [[1, k2], [0, EMB]])
                nc.vector.tensor_tensor(gh, gh, alpha_b, OP.mult)
                h2 = sp2.tile([128, EMB], F32, tag="h2")
                g_fk = apv(G[:], [[1, EMB], [TB2, k2]])
                nc.vector.reduce_sum(h2[:], g_fk, axis=AX.X)

                nc.vector.tensor_tensor(h2[:], h2[:], cb["B2"][:], OP.add)
                ms = sp2.tile([128, 1], F32, tag="ms2")
                nc.vector.reduce_sum(ms[:], h2[:], axis=AX.X)
                mu = sp2.tile([128, 1], F32, tag="mu2")
                nc.scalar.activation(mu[:], ms[:], ACT.Copy, scale=1.0 / EMB)
                nc.vector.tensor_scalar_sub(h2[:], h2[:], mu[:])
                sq = sp2.tile([128, EMB], F32, tag="sq2")
                nc.scalar.activation(sq[:], h2[:], ACT.Square)
                var = sp2.tile([128, 1], F32, tag="var2")
                nc.vector.reduce_sum(var[:], sq[:], axis=AX.X)
                std = sp2.tile([128, 1], F32, tag="std2")
                nc.scalar.activation(std[:], var[:], ACT.Sqrt, bias=epst[:],
                                     scale=1.0 / EMB)
                rstd = sp2.tile([128, 1], F32, tag="rstd2")
                nc.vector.reciprocal(rstd[:], std[:])
                nc.vector.tensor_scalar_mul(h2[:], h2[:], rstd[:])
                nc.vector.tensor_tensor(h2[:], h2[:], cb["G2"][:], OP.mult)
                nc.vector.tensor_tensor(h2[:], h2[:], cb["Be2"][:], OP.add)
                nc.sync.dma_start(OUT[t * 128:(t + 1) * 128, :], h2[:])

    nc.compile()
    return nc


# ------------------------------------------------------------------ execution
def make_in_maps(cfg, prep, inputs):
    wts = host_weights(cfg, inputs)
    xt = host_xt(cfg, prep, inputs["x"])
    in_maps = []
    for c in range(NCORE):
        m = {
            "xt": xt,
            "w1ext": wts["W1ext"], "w2ext": wts["W2ext"],
            "idx16": np.ascontiguousarray(prep.idx16[c]),
        }
        for nm in ["B1", "G1", "Be1", "B2", "G2", "Be2"]:
            m[nm.lower()] = wts[nm]
        in_maps.append(m)
    return in_maps


def assemble(cfg, prep, outs):
    full = np.zeros((cfg.N, EMB), np.float32)
    for c in range(NCORE):
        o = outs[c]["out"]
        olds = prep.new2old[c * cfg.NPC:(c + 1) * cfg.NPC]
        valid = olds >= 0
        full[olds[valid]] = o[valid]
    return full


_CACHE = {}


def kernel(**inputs):
    from concourse.bass_utils import run_bass_kernel_spmd
    cfg = CFG
    edge_index = np.asarray(inputs["edge_index"])
    if "prog" not in _CACHE:
        prep = host_prep(cfg, edge_index)
        nc = build_program(cfg, prep)
        _CACHE["prog"] = (prep, nc)
    prep, nc = _CACHE["prog"]
    in_maps = make_in_maps(cfg, prep, inputs)
    res = run_bass_kernel_spmd(
        nc, in_maps, core_ids=list(range(NCORE)),
        trace=bool(int(os.environ.get("GAT_TRACE", "0"))))
    out = assemble(cfg, prep, res.results)
    if res.exec_time_ns is not None:
        kernel.last_exec_time_ns = res.exec_time_ns
    return out


kernel.last_exec_time_ns = None



# revision 14
# speedup vs baseline: 2.0894x; 2.0894x over previous
"""2-layer GAT (GATConv x2 + LayerNorm + ReLU) on Trainium2, 8-core SPMD.

v2 design (vs baseline): pair-gather edge streams.
  - Nodes degree-sorted (incl self loop), dealt round-robin across 8 cores;
    each core owns NPC=6272 dst slots (49 tiles x 128).
  - Per layer a packed bf16 node table in DRAM:
      layer 1: row = [h(128) | a_s(4) | a_d(4) | pad] -> 192 cols (384B)
      layer 2: row = [h2(64) | a_s2(1) | a_d2(1) | pad] -> 128 cols (256B)
    Rows are gathered in PAIRS (idx = row>>1, elem = 2 rows) so a single
    int16 gather covers all 50176 rows (25088 pair indices < 32768): ONE
    dma_gather per dst tile per layer, real edges only (~1.06x padding
    vs 1.86x for the per-half slot-padded baseline).
  - Per dst tile [128 dst x K slots x 2 pair-members]: slot 0 is the self
    loop; host-built bf16 masks (-1e9 on the wrong pair member / pad slots)
    zero out garbage in the softmax.  e-scores are bounded (|e| <= ~6) so
    the segment max is skipped; weights w = exp(leaky(e))/den are folded
    into the gathered h before a single strided DVE reduction.
  - Layer-2 tables AllGathered in 4 chunks (Shared scratchpad output)
    overlapped with the layer-1 tail.
"""

import os
import types
from contextlib import ExitStack

import numpy as np

import concourse.bass as bass
import concourse.mybir as mybir
import concourse.tile as tile
from concourse import bacc
from concourse.bass import AP
from concourse.masks import make_identity

F32 = mybir.dt.float32
BF16 = mybir.dt.bfloat16
I16 = mybir.dt.int16
AX = mybir.AxisListType
OP = mybir.AluOpType
ACT = mybir.ActivationFunctionType

# ---------------------------------------------------------------- problem cfg
N = 50000
E = 800000
IN_DIM = 128
HID = 32
HEADS = 4
EMB = 64
NEG = 0.2
EPS = 1e-5
NCORE = 8
ROW1 = 192        # layer-1 packed table row (bf16 cols): h(128) as(4) ad(4) pad
ROW2 = 128        # layer-2 packed table row: h2(64) as(1) ad(1) pad
NEGBIG = -1e9
AG_CHUNKS = 4


def make_cfg(n_nodes, tiles_per_core):
    c = types.SimpleNamespace()
    c.N = n_nodes
    c.TILES = tiles_per_core
    c.NPC = tiles_per_core * 128
    c.NPAD = NCORE * c.NPC
    c.NPAIR = c.NPAD // 2
    c.REAL_PC = n_nodes // NCORE
    # AllGather chunk geometry (tile index ranges per chunk)
    ends = [((i + 1) * tiles_per_core) // AG_CHUNKS for i in range(AG_CHUNKS)]
    starts = [0] + ends[:-1]
    c.AG_T0 = starts
    c.AG_T1 = ends
    # table rows are stored [chunk, core, rows] so each chunked AllGather
    # output region is contiguous; srow maps (core, pos) -> storage row
    rs = [s * 128 for s in starts]
    re = [e * 128 for e in ends]
    c.CH_ROWS = [b - a for a, b in zip(rs, re)]
    c.CH_BASE = [NCORE * a for a in rs]
    pos = np.arange(c.NPC)
    k = np.searchsorted(np.asarray(re), pos, side="right")
    c.SROW = np.stack([
        np.asarray(c.CH_BASE)[k] + cc * np.asarray(c.CH_ROWS)[k]
        + (pos - np.asarray(rs)[k]) for cc in range(NCORE)])  # [core, pos]
    assert c.REAL_PC % 2 == 0 and c.REAL_PC < c.NPC and c.NPAIR <= 32768
    return c


CFG = make_cfg(N, 49)


# ------------------------------------------------------------------ host prep
def host_prep(cfg, edge_index):
    """Node permutation, per-tile slot counts, idx16 pair stream and masks.

    Per (core, tile t) the gather stream is K[t] columns of 128 slots:
    column 0 = self loops, columns 1.. = neighbors (dst-grouped), pads point
    at the PADPAIR rows.  idx values are PAIR indices (row>>1); the bf16
    mask [128, 2*K[t]] holds 0 on the real pair member and -1e9 elsewhere.
    """
    n, npc, tiles = cfg.N, cfg.NPC, cfg.TILES
    src = np.asarray(edge_index[0], np.int64)
    dst = np.asarray(edge_index[1], np.int64)

    deg = np.bincount(dst, minlength=n) + 1           # incl self loop
    order = np.argsort(-deg, kind="stable")
    newid = np.empty(n, np.int64)
    r = np.arange(n)
    newid[order] = (r % NCORE) * npc + (r // NCORE)
    new2old = np.full(NCORE * npc, -1, np.int64)
    new2old[newid] = np.arange(n)

    degs_sorted = np.zeros(tiles * 1024, np.int64)
    degs_sorted[:n] = deg[order]
    K = degs_sorted.reshape(tiles, 1024).max(axis=1)
    K = np.maximum(K, 1).astype(np.int64)             # slots per tile

    IOFS = np.r_[0, np.cumsum(K)] * 128               # idx stream offsets
    MOFS = np.r_[0, np.cumsum(K)] * 2                 # mask col offsets
    total_idx = int(IOFS[-1])
    mtot = int(MOFS[-1])

    tl = cfg.SROW.reshape(-1)              # logical row -> storage row
    padpair = int(tl[cfg.REAL_PC]) >> 1    # core-0 pad rows (zero features)
    assert int(tl[cfg.REAL_PC]) % 2 == 0

    ns, nd = newid[src], newid[dst]
    eo = np.argsort(nd, kind="stable")
    snd, sns = nd[eo], ns[eo]
    starts = np.r_[0, np.flatnonzero(np.diff(snd)) + 1]
    runlen = np.diff(np.r_[starts, len(snd)])
    runpos = np.arange(len(snd)) - np.repeat(starts, runlen)
    slot = runpos + 1                                  # col 0 = self
    cs = snd // npc
    pos = snd % npc
    ts_ = pos // 128
    ps_ = pos % 128
    assert (slot < K[ts_]).all()

    srows = tl[sns]
    idx16 = np.full((NCORE, total_idx), padpair, np.int16)
    mask = np.full((NCORE, 128, mtot), NEGBIG, np.float32)
    posi = IOFS[ts_] + slot * 128 + ps_
    idx16[cs, posi] = (srows >> 1).astype(np.int16)
    mask[cs, ps_, MOFS[ts_] + slot * 2 + (srows & 1)] = 0.0

    # self column (col 0) per core
    for c in range(NCORE):
        own = c * npc + np.arange(npc)
        valid = new2old[own] >= 0
        t_all = np.arange(npc) // 128
        p_all = np.arange(npc) % 128
        sown = tl[own]
        idx16[c, IOFS[t_all[valid]] + p_all[valid]] = \
            (sown[valid] >> 1).astype(np.int16)
        mask[c, p_all[valid], MOFS[t_all[valid]] + (sown[valid] & 1)] = 0.0

    w = idx16.reshape(NCORE, total_idx // 16, 16).transpose(0, 2, 1)
    idx16_w = np.ascontiguousarray(np.tile(w, (1, 8, 1)))
    return types.SimpleNamespace(
        new2old=new2old, newid=newid,
        K=[int(v) for v in K], IOFS=[int(v) for v in IOFS // 16],
        MOFS=[int(v) for v in MOFS],
        c16=total_idx // 16, mtot=mtot,
        idx16=idx16_w, mask=_bf16(mask),
    )


def _bf16(x):
    import ml_dtypes
    x = np.ascontiguousarray(np.asarray(x, np.float32))
    u = x.view(np.uint32)
    r = ((u + 0x7FFF + ((u >> 16) & 1)) >> 16).astype(np.uint16)
    return r.view(ml_dtypes.bfloat16)


def host_weights(cfg, inputs):
    W1 = np.asarray(inputs["W1"], np.float32)
    W2 = np.asarray(inputs["W2"], np.float32)
    as1 = np.asarray(inputs["att_src1"], np.float32)
    ad1 = np.asarray(inputs["att_dst1"], np.float32)
    as2 = np.asarray(inputs["att_src2"], np.float32)
    ad2 = np.asarray(inputs["att_dst2"], np.float32)
    W1r = W1.reshape(IN_DIM, HEADS, HID)
    w_as1 = np.einsum("fhc,hc->fh", W1r, as1)
    w_ad1 = np.einsum("fhc,hc->fh", W1r, ad1)
    W1ext = np.concatenate([W1, w_as1, w_ad1], axis=1)            # [128,136]
    W2ext = np.concatenate([W2, W2 @ as2[0][:, None], W2 @ ad2[0][:, None]],
                           axis=1)                                # [128,66]
    par0 = ((np.arange(128) + 1) % 2).astype(np.float32)[:, None]
    par1 = (np.arange(128) % 2).astype(np.float32)[:, None]
    return {
        "w1ext": _bf16(W1ext), "w2ext": _bf16(W2ext),
        "b1": np.tile(np.asarray(inputs["b1"], np.float32), (128, 1)),
        "g1": np.tile(np.asarray(inputs["gamma1"], np.float32), (128, 1)),
        "be1": np.tile(np.asarray(inputs["beta1"], np.float32), (128, 1)),
        "b2": np.tile(np.asarray(inputs["b2"], np.float32), (128, 1)),
        "g2": np.tile(np.asarray(inputs["gamma2"], np.float32), (128, 1)),
        "be2": np.tile(np.asarray(inputs["beta2"], np.float32), (128, 1)),
        "par0": par0, "par1": par1,
    }


def host_xt(cfg, prep, x):
    tl = cfg.SROW.reshape(-1)
    xt = np.zeros((IN_DIM, cfg.NPAD), np.float32)
    xt[:, tl[prep.newid]] = np.asarray(x, np.float32).T
    return _bf16(xt)


# ----------------------------------------------------------------- AP helpers
def apv(ap: AP, dims, extra_offset=0):
    """Replace the free dims of `ap` with explicit [step, count] pairs."""
    return AP(ap.tensor, int(ap.offset + extra_offset),
              [list(ap.ap[0])] + [[int(s), int(n)] for s, n in dims])


def apd(ap: AP, dims, extra_offset=0):
    """DRAM AP with explicit dims (no partition dim)."""
    return AP(ap.tensor, int(ap.offset + extra_offset),
              [[int(s), int(n)] for s, n in dims])


# ------------------------------------------------------------- device program
def build_program(cfg, prep):
    nc = bacc.Bacc("TRN2", target_bir_lowering=False, debug=False,
                   num_devices=NCORE)
    tiles, npc, npad = cfg.TILES, cfg.NPC, cfg.NPAD
    K, IOFS, MOFS = prep.K, prep.IOFS, prep.MOFS
    FB1 = IN_DIM + 2 * HEADS            # 136
    FB2 = EMB + 2                       # 66

    XT = nc.dram_tensor("xt", [IN_DIM, npad], BF16, kind="ExternalInput")
    W1e = nc.dram_tensor("w1ext", [IN_DIM, FB1], BF16, kind="ExternalInput")
    W2e = nc.dram_tensor("w2ext", [IN_DIM, FB2], BF16, kind="ExternalInput")
    IDX16 = nc.dram_tensor("idx16", [128, prep.c16], I16, kind="ExternalInput")
    MASK = nc.dram_tensor("mask", [128, prep.mtot], BF16, kind="ExternalInput")
    CB = {}
    for nm, cols in [("b1", IN_DIM), ("g1", IN_DIM), ("be1", IN_DIM),
                     ("b2", EMB), ("g2", EMB), ("be2", EMB),
                     ("par0", 1), ("par1", 1)]:
        CB[nm] = nc.dram_tensor(nm, [128, cols], F32, kind="ExternalInput")
    OUT = nc.dram_tensor("out", [npc, EMB], F32, kind="ExternalOutput")

    with tile.TileContext(nc, num_cores=NCORE) as tc, ExitStack() as ctx:
        dram = ctx.enter_context(tc.tile_pool(name="dram", bufs=1,
                                              space="DRAM"))
        t1b = dram.tile([npad, ROW1], BF16, name="t1b")
        t2sh = dram.tile([npc, ROW2], BF16, name="t2sh")
        t2b = dram.tile([npad, ROW2], BF16, name="t2b")

        cpool = ctx.enter_context(tc.tile_pool(name="const", bufs=1))
        w1s = cpool.tile([IN_DIM, FB1], BF16, name="w1s")
        w2s = cpool.tile([IN_DIM, FB2], BF16, name="w2s")
        nc.sync.dma_start(w1s[:], W1e[:])
        nc.sync.dma_start(w2s[:], W2e[:])
        cb = {}
        for nm in CB:
            cb[nm] = cpool.tile(list(CB[nm].shape), F32, name=f"sb_{nm}")
            nc.sync.dma_start(cb[nm][:], CB[nm][:])
        ident = cpool.tile([128, 128], F32, name="ident")
        make_identity(nc, ident[:])
        epst = cpool.tile([128, 1], F32, name="epst")
        nc.vector.memset(epst[:], EPS)
        i16b = cpool.tile([128, prep.c16], I16, name="i16b")
        nc.sync.dma_start(i16b[:], IDX16[:])
        mkb = cpool.tile([128, prep.mtot], BF16, name="mkb")
        nc.sync.dma_start(mkb[:], MASK[:])

        # ---------------- phase 1: layer-1 packed table, replicated
        GRP = 4
        ngrp = npad // (128 * GRP)
        with tc.tile_pool(name="ph1", bufs=3) as ph1, \
             tc.tile_pool(name="ph1p", bufs=8, space="PSUM") as ph1p:
            for g in range(ngrp):
                xsl = ph1.tile([128, 128 * GRP], BF16, tag="xsl")
                nc.sync.dma_start(xsl[:],
                                  XT[:, g * 128 * GRP:(g + 1) * 128 * GRP])
                stage = ph1.tile([128, GRP, FB1], BF16, tag="stage")
                for s in range(GRP):
                    ps = ph1p.tile([128, FB1], F32, tag="ps")
                    nc.tensor.matmul(ps[:], lhsT=xsl[:, s * 128:(s + 1) * 128],
                                     rhs=w1s[:], start=True, stop=True)
                    nc.vector.tensor_copy(stage[:, s, :], ps[:])
                rows = slice(g * 128 * GRP, (g + 1) * 128 * GRP)
                nc.sync.dma_start(
                    t1b[rows, 0:FB1].rearrange("(s p) c -> p s c", p=128),
                    stage[:, :, :])
        # (no pad-row fixups needed: pad slots are killed by the -1e9 masks)

        t1pairs = apd(t1b[:], [[2 * ROW1, cfg.NPAIR], [1, 2 * ROW1]])
        t2pairs = apd(t2b[:], [[2 * ROW2, cfg.NPAIR], [1, 2 * ROW2]])

        # AllGather chunk boundaries (tile indices)
        ag_after = [e - 1 for e in cfg.AG_T1]

        # ---------------- phase 2: layer-1 tiles -> t2sh shard + chunked AG
        with tc.tile_pool(name="gp", bufs=3) as gp, \
             tc.tile_pool(name="sp", bufs=3) as sp, \
             tc.tile_pool(name="pp", bufs=4, space="PSUM") as pp:
            for t in range(tiles):
                S = K[t]
                K2 = 2 * S
                G = gp.tile([128, S, 2, ROW1], BF16, tag="G")
                nc.gpsimd.dma_gather(
                    apv(G[:], [[2 * ROW1, S], [1, 2 * ROW1]]), t1pairs,
                    i16b[:, IOFS[t]:IOFS[t] + S * 8],
                    S * 128, S * 128, 2 * ROW1, single_packet=False)

                # a_d[dst] from self column (slot 0), parity-selected
                ad0 = sp.tile([128, HEADS], F32, tag="ad0")
                nc.vector.tensor_scalar(ad0[:], G[:, 0, 0, IN_DIM + HEADS:FB1],
                                        cb["par0"][:], None, OP.mult)
                ad1 = sp.tile([128, HEADS], F32, tag="ad1")
                nc.vector.tensor_scalar(ad1[:], G[:, 0, 1, IN_DIM + HEADS:FB1],
                                        cb["par1"][:], None, OP.mult)
                ad = sp.tile([128, HEADS], F32, tag="ad")
                nc.vector.tensor_tensor(ad[:], ad0[:], ad1[:], OP.add)

                # e = a_s[src] + a_d[dst] + mask ; leaky
                e = sp.tile([128, K2, HEADS], F32, tag="e")
                as_v = apv(G[:], [[ROW1, K2], [1, HEADS]], IN_DIM)
                ad_v = apv(ad[:], [[0, K2], [1, HEADS]])
                nc.vector.tensor_tensor(e[:], as_v, ad_v, OP.add)
                mk_v = apv(mkb[:], [[1, K2], [0, HEADS]], MOFS[t])
                nc.vector.tensor_tensor(e[:], e[:], mk_v, OP.add)
                e2 = sp.tile([128, K2, HEADS], F32, tag="e2")
                nc.vector.tensor_scalar(e2[:], e[:], NEG, None, OP.mult)
                nc.vector.tensor_tensor(e[:], e[:], e2[:], OP.max)
                # ex = exp(e); den per head; w = ex/den
                nc.scalar.activation(e[:], e[:], ACT.Exp)
                den = sp.tile([128, HEADS], F32, tag="den")
                nc.vector.reduce_sum(den[:], apv(e[:], [[1, HEADS], [HEADS, K2]]),
                                     axis=AX.X)
                nc.vector.tensor_scalar(den[:], den[:], 1e-20, None, OP.add)
                inv = sp.tile([128, HEADS], F32, tag="inv")
                nc.vector.reciprocal(inv[:], den[:])
                wb = sp.tile([128, K2, HEADS], BF16, tag="wb")
                inv_v = apv(inv[:], [[0, K2], [1, HEADS]])
                nc.vector.tensor_tensor(wb[:], e[:], inv_v, OP.mult)

                # fold w into gathered h (per member), then aggregate
                for m in range(2):
                    gm = apv(G[:], [[2 * ROW1, S], [1, IN_DIM]], m * ROW1)
                    wm = apv(wb[:], [[2 * HEADS, S], [1, HEADS], [0, HID]],
                             m * HEADS)
                    nc.vector.tensor_tensor(gm, gm, wm, OP.mult)
                h1 = sp.tile([128, IN_DIM], F32, tag="h1")
                nc.vector.reduce_sum(h1[:], apv(G[:], [[1, IN_DIM], [ROW1, K2]]),
                                     axis=AX.X)

                # + b1, layernorm, relu
                nc.vector.tensor_tensor(h1[:], h1[:], cb["b1"][:], OP.add)
                ms = sp.tile([128, 1], F32, tag="ms")
                nc.vector.reduce_sum(ms[:], h1[:], axis=AX.X)
                mu = sp.tile([128, 1], F32, tag="mu")
                nc.vector.tensor_scalar(mu[:], ms[:], 1.0 / IN_DIM, None,
                                        OP.mult)
                nc.vector.tensor_scalar(h1[:], h1[:], mu[:], None, OP.subtract)
                sq = sp.tile([128, IN_DIM], F32, tag="sq")
                nc.vector.tensor_tensor(sq[:], h1[:], h1[:], OP.mult)
                var = sp.tile([128, 1], F32, tag="var")
                nc.vector.reduce_sum(var[:], sq[:], axis=AX.X)
                std = sp.tile([128, 1], F32, tag="std")
                nc.scalar.activation(std[:], var[:], ACT.Sqrt, bias=epst[:],
                                     scale=1.0 / IN_DIM)
                rstd = sp.tile([128, 1], F32, tag="rstd")
                nc.vector.reciprocal(rstd[:], std[:])
                nc.vector.tensor_scalar(h1[:], h1[:], rstd[:], None, OP.mult)
                nc.vector.tensor_tensor(h1[:], h1[:], cb["g1"][:], OP.mult)
                nc.vector.tensor_tensor(h1[:], h1[:], cb["be1"][:], OP.add)
                nc.vector.tensor_scalar(h1[:], h1[:], 0.0, None, OP.max)

                # layer-2 packed shard rows: h1 @ W2ext via PE transpose
                pst = pp.tile([128, 128], F32, tag="pst")
                nc.tensor.transpose(pst[:], h1[:], ident[:])
                h1t = sp.tile([128, 128], BF16, tag="h1t")
                nc.vector.tensor_copy(h1t[:], pst[:])
                ps2 = pp.tile([128, FB2], F32, tag="ps2")
                nc.tensor.matmul(ps2[:], lhsT=h1t[:], rhs=w2s[:],
                                 start=True, stop=True)
                t2row = sp.tile([128, FB2], BF16, tag="t2row")
                nc.vector.tensor_copy(t2row[:], ps2[:])
                nc.sync.dma_start(t2sh[t * 128:(t + 1) * 128, 0:FB2], t2row[:])

                if t in ag_after:
                    ci = ag_after.index(t)
                    r0 = cfg.AG_T0[ci] * 128
                    r1 = (t + 1) * 128
                    ag_out = apd(t2b[:],
                                 [[1, NCORE * (r1 - r0) * ROW2]],
                                 cfg.CH_BASE[ci] * ROW2)
                    nc.gpsimd.collective_compute(
                        "AllGather", OP.bypass,
                        replica_groups=[list(range(NCORE))],
                        ins=[t2sh[r0:r1, :].opt()], outs=[ag_out.opt()])

        # ---------------- phase 3: layer 2
        with tc.tile_pool(name="gp2", bufs=3) as gp2, \
             tc.tile_pool(name="sp2", bufs=3) as sp2:
            for t in range(tiles):
                S = K[t]
                K2 = 2 * S
                G2 = gp2.tile([128, S, 2, ROW2], BF16, tag="G2")
                nc.gpsimd.dma_gather(
                    apv(G2[:], [[2 * ROW2, S], [1, 2 * ROW2]]), t2pairs,
                    i16b[:, IOFS[t]:IOFS[t] + S * 8],
                    S * 128, S * 128, 2 * ROW2, single_packet=False)

                ad0 = sp2.tile([128, 1], F32, tag="ad0_2")
                nc.vector.tensor_scalar(ad0[:], G2[:, 0, 0, EMB + 1:EMB + 2],
                                        cb["par0"][:], None, OP.mult)
                ad1 = sp2.tile([128, 1], F32, tag="ad1_2")
                nc.vector.tensor_scalar(ad1[:], G2[:, 0, 1, EMB + 1:EMB + 2],
                                        cb["par1"][:], None, OP.mult)
                ad = sp2.tile([128, 1], F32, tag="ad_2")
                nc.vector.tensor_tensor(ad[:], ad0[:], ad1[:], OP.add)

                e = sp2.tile([128, K2], F32, tag="e_2")
                nc.vector.tensor_scalar(e[:], apv(G2[:], [[ROW2, K2]], EMB),
                                        ad[:], None, OP.add)
                nc.vector.tensor_tensor(e[:], e[:],
                                        apv(mkb[:], [[1, K2]], MOFS[t]), OP.add)
                e2 = sp2.tile([128, K2], F32, tag="e2_2")
                nc.vector.tensor_scalar(e2[:], e[:], NEG, None, OP.mult)
                nc.vector.tensor_tensor(e[:], e[:], e2[:], OP.max)
                nc.scalar.activation(e[:], e[:], ACT.Exp)
                den = sp2.tile([128, 1], F32, tag="den2")
                nc.vector.reduce_sum(den[:], e[:], axis=AX.X)
                nc.vector.tensor_scalar(den[:], den[:], 1e-20, None, OP.add)
                inv = sp2.tile([128, 1], F32, tag="inv2")
                nc.vector.reciprocal(inv[:], den[:])
                wb = sp2.tile([128, K2], BF16, tag="wb2")
                nc.vector.tensor_scalar(wb[:], e[:], inv[:], None, OP.mult)

                for m in range(2):
                    gm = apv(G2[:], [[2 * ROW2, S], [1, EMB]], m * ROW2)
                    wm = apv(wb[:], [[2, S], [0, EMB]], m)
                    nc.vector.tensor_tensor(gm, gm, wm, OP.mult)
                h2 = sp2.tile([128, EMB], F32, tag="h2")
                nc.vector.reduce_sum(h2[:], apv(G2[:], [[1, EMB], [ROW2, K2]]),
                                     axis=AX.X)

                nc.vector.tensor_tensor(h2[:], h2[:], cb["b2"][:], OP.add)
                ms = sp2.tile([128, 1], F32, tag="ms2")
                nc.vector.reduce_sum(ms[:], h2[:], axis=AX.X)
                mu = sp2.tile([128, 1], F32, tag="mu2")
                nc.vector.tensor_scalar(mu[:], ms[:], 1.0 / EMB, None, OP.mult)
                nc.vector.tensor_scalar(h2[:], h2[:], mu[:], None, OP.subtract)
                sq = sp2.tile([128, EMB], F32, tag="sq2")
                nc.vector.tensor_tensor(sq[:], h2[:], h2[:], OP.mult)
                var = sp2.tile([128, 1], F32, tag="var2")
                nc.vector.reduce_sum(var[:], sq[:], axis=AX.X)
                std = sp2.tile([128, 1], F32, tag="std2")
                nc.scalar.activation(std[:], var[:], ACT.Sqrt, bias=epst[:],
                                     scale=1.0 / EMB)
                rstd = sp2.tile([128, 1], F32, tag="rstd2")
                nc.vector.reciprocal(rstd[:], std[:])
                nc.vector.tensor_scalar(h2[:], h2[:], rstd[:], None, OP.mult)
                nc.vector.tensor_tensor(h2[:], h2[:], cb["g2"][:], OP.mult)
                nc.vector.tensor_tensor(h2[:], h2[:], cb["be2"][:], OP.add)
                nc.sync.dma_start(OUT[t * 128:(t + 1) * 128, :], h2[:])

    nc.compile()
    return nc


# ------------------------------------------------------------------ execution
def make_in_maps(cfg, prep, inputs):
    wts = host_weights(cfg, inputs)
    xt = host_xt(cfg, prep, inputs["x"])
    in_maps = []
    for c in range(NCORE):
        m = {"xt": xt,
             "idx16": prep.idx16[c],
             "mask": prep.mask[c]}
        m.update(wts)
        in_maps.append(m)
    return in_maps


def assemble(cfg, prep, outs):
    full = np.zeros((cfg.N, EMB), np.float32)
    for c in range(NCORE):
        o = outs[c]["out"]
        olds = prep.new2old[c * cfg.NPC:(c + 1) * cfg.NPC]
        valid = olds >= 0
        full[olds[valid]] = o[valid]
    return full


_CACHE = {}


def kernel(**inputs):
    from concourse.bass_utils import run_bass_kernel_spmd
    cfg = CFG
    edge_index = np.asarray(inputs["edge_index"])
    if "prog" not in _CACHE:
        prep = host_prep(cfg, edge_index)
        nc = build_program(cfg, prep)
        _CACHE["prog"] = (prep, nc)
    prep, nc = _CACHE["prog"]
    in_maps = make_in_maps(cfg, prep, inputs)
    res = run_bass_kernel_spmd(
        nc, in_maps, core_ids=list(range(NCORE)),
        trace=bool(int(os.environ.get("GAT_TRACE", "0"))))
    out = assemble(cfg, prep, res.results)
    if res.exec_time_ns is not None:
        kernel.last_exec_time_ns = res.exec_time_ns
    return out


kernel.last_exec_time_ns = None
